# revision 3
# baseline (speedup 1.0000x reference)
"""Trainium2 Bass kernel for nn_Attention_9431748182617.

Quirky attention: scores z[b,k,q] = (q_h . k_h) / sqrt(D), softmax over the
QUERY axis (per key row), out[q] = sum_k A[k,q] * v[k], then output projection.

Sharding (8 NeuronCores):
  - tensor-parallel over heads: 16 heads -> 2 heads per core.
    Each core owns rows [128c, 128c+128) of Wq/Wk/Wv (its 2 heads) and
    computes q/k/v + attention for those heads over the full batch.
  - z^T (local 128 rows of L, all of B*S) is AllGather'd per batch.
  - output projection sharded by output feature D: core c computes
    out^T rows [128c, 128c+128) using Wo^T[:, 128c:128c+128] for ALL s.
  - host concatenates the 8 out^T blocks and transposes.

The Scalar engine (exp over B*H*S^2/8 = 33.5M elems/core) is the critical
resource; all other work is issued as <=8-matmul chunks interleaved between
score units so ScalarE never stalls.  A tunable subset of softmax units runs
as (1+s/2)^2 on DVE/Pool (scores are |s|<0.7 so the quadratic is ~0.4% err)
with the denominator fused into the last op's accumulator, offloading ACT.
Matmuls in bf16 (fp32 PSUM accumulation), Q/K projections fp8 DoubleRow.
"""

import os
from collections import defaultdict

import numpy as np
import ml_dtypes

import concourse.bass as bass
import concourse.mybir as mybir
import concourse.tile as tile
from concourse.bass_utils import run_bass_kernel_spmd
from concourse.masks import make_identity

B, S, D = 4, 2048, 1024
L, H = 1024, 16
DH = L // H               # 64
NCORES = 8
LPC = L // NCORES         # 128 l-rows (= 2 heads) per core
DPC = D // NCORES         # 128 out-feature rows per core
SCALE = 1.0 / (D ** 0.5)
KC = S // 128             # 16 key chunks of 128
BF16 = mybir.dt.bfloat16
F32 = mybir.dt.float32
F8 = mybir.dt.float8e4
EXP = mybir.ActivationFunctionType.Exp
MULT = mybir.AluOpType.mult
ADD = mybir.AluOpType.add

LAST_EXEC_NS = None

# softmax-unit engine assignment: for each (kc, h, half) unit, which engine
# computes the exponentials. "a"=ScalarE exp, "v"=DVE (1+s/2)^2, "p"=Pool.
# 64 units per batch; same pattern every batch.
UNIT_ENGINE = {}


def _default_unit_plan(n_dve=0, n_pool=0):
    """Distribute n_dve DVE units + n_pool Pool units evenly over the
    64 (kc, h, half) units of a batch; the rest stay on ScalarE."""
    plan = {}
    units = [(kc, h, half) for kc in range(KC) for h in range(2)
             for half in range(2)]
    n = len(units)
    picks = {}
    for cnt, tag in ((n_dve, "v"), (n_pool, "p")):
        step = n / max(cnt, 1)
        k = 0.5
        placed = 0
        while placed < cnt:
            i = int(k) % n
            while units[i] in picks:
                i = (i + 1) % n
            picks[units[i]] = tag
            placed += 1
            k += step
    for u in units:
        plan[u] = picks.get(u, "a")
    return plan


def _body(tc, xT, x8, wq8, wk8, wvT, woT, outT, zloc, zfull, plan):
    nc = tc.nc
    from contextlib import ExitStack

    with ExitStack() as ctx:
        const = ctx.enter_context(tc.tile_pool(name="const", bufs=1))
        xpool = ctx.enter_context(tc.tile_pool(name="xpool", bufs=1))
        qk = ctx.enter_context(tc.tile_pool(name="qk", bufs=2))
        vtpool = ctx.enter_context(tc.tile_pool(name="vtpool", bufs=1))
        vpool = ctx.enter_context(tc.tile_pool(name="vpool", bufs=2))
        apool = ctx.enter_context(tc.tile_pool(name="apool", bufs=7))
        tpool = ctx.enter_context(tc.tile_pool(name="tpool", bufs=4))
        small = ctx.enter_context(tc.tile_pool(name="small", bufs=8))
        ztp = ctx.enter_context(tc.tile_pool(name="ztp", bufs=2))
        zslab = ctx.enter_context(tc.tile_pool(name="zslab", bufs=2))
        osb_p = ctx.enter_context(tc.tile_pool(name="osb_p", bufs=2))
        # all 8 PSUM banks in one 4-deep [128,1024] pool: scores, AV
        # partials, projections, out-projection all cycle through it
        ps = ctx.enter_context(tc.tile_pool(name="ps", bufs=1, space="PSUM"))

        # ---- constants: weights ----
        wq_sb = const.tile([128, 4, 2, 128], F8, name="wq_sb")
        wk_sb = const.tile([128, 4, 2, 128], F8, name="wk_sb")
        nc.sync.dma_start(wq_sb, wq8)
        nc.sync.dma_start(wk_sb, wk8)
        wv_sb = const.tile([128, 8, 128], BF16, name="wv_sb")
        wo_sb = const.tile([128, 8, 128], BF16, name="wo_sb")
        for dc in range(8):
            nc.sync.dma_start(wv_sb[:, dc, :], wvT[dc * 128:(dc + 1) * 128, :])
            nc.sync.dma_start(wo_sb[:, dc, :], woT[dc * 128:(dc + 1) * 128, :])
        # fire the exp table load (~2.7us) under the startup DMAs
        warm_in = const.tile([128, 1], F32, name="warm_in")
        warm_out = const.tile([128, 1], F32, name="warm_out")
        nc.vector.memset(warm_in, 0.0)
        nc.scalar.activation(warm_out, warm_in, EXP)

        def load_x8(b):
            x8_c = []
            for j in range(4):
                xc = xpool.tile([128, 2, S], F8, name=f"x8c{j}", tag=f"x8{j}")
                nc.gpsimd.dma_start(xc, x8[b, j])
                x8_c.append(xc)
            return x8_c

        def load_xT(b):
            x_c = []
            for dc in range(8):
                xc = xpool.tile([128, S], BF16, name=f"xc{dc}", tag=f"x{dc}")
                nc.gpsimd.dma_start(xc, xT[b, dc * 128:(dc + 1) * 128, :])
                x_c.append(xc)
            return x_c

        def proj_w8_half(w_sb, nm, x8_c, half, dest):
            """One s-half of a Q/K projection in fp8 DoubleRow (8 MMs)."""
            pw = ps.tile([128, 1024], F32, name="pw8", tag="work", bufs=4)
            for j in range(4):
                for q in range(2):
                    sc = half * 2 + q
                    nc.tensor.matmul(
                        pw[:, q * 512:(q + 1) * 512],
                        lhsT=w_sb[:, j, :, :],
                        rhs=x8_c[j][:, :, sc * 512:(sc + 1) * 512],
                        start=(j == 0),
                        stop=(j == 3),
                        perf_mode=mybir.MatmulPerfMode.DoubleRow,
                    )
            nc.vector.tensor_copy(dest[:, half * 1024:(half + 1) * 1024], pw)

        def proj_w_half(w_sb, x_c, half, dest, part, pw_box):
            """Half of the V projection, split into two 8-MM parts sharing
            one PSUM tile (part 0 allocates, part 1 copies out)."""
            if part == 0:
                pw_box[half] = ps.tile([128, 1024], F32, name="pwv",
                                       tag="work", bufs=4)
            pw = pw_box[half]
            for dc in range(part * 4, part * 4 + 4):
                for q in range(2):
                    sc = half * 2 + q
                    nc.tensor.matmul(
                        pw[:, q * 512:(q + 1) * 512],
                        lhsT=w_sb[:, dc, :],
                        rhs=x_c[dc][:, sc * 512:(sc + 1) * 512],
                        start=(dc == 0),
                        stop=(dc == 7),
                    )
            if part == 1:
                nc.vector.tensor_copy(dest[:, half * 1024:(half + 1) * 1024],
                                      pw)

        def transpose_v_chunk(vt, v_sb, c0, c1):
            for c in range(c0, c1):
                nc.sync.dma_start_transpose(
                    v_sb[:, c, :], vt[:, c * 128:(c + 1) * 128])

        def proj(b):
            """Initial (unhooked) projection burst for batch 0."""
            x8_c = load_x8(b)
            x_c = load_xT(b)
            qt = qk.tile([128, S], BF16, name="qt", tag="qt")
            kt = qk.tile([128, S], BF16, name="kt", tag="kt")
            vt = vtpool.tile([128, S], BF16, name="vt", tag="vt")
            for half in range(2):
                proj_w8_half(wq_sb, "qt", x8_c, half, qt)
            for half in range(2):
                proj_w8_half(wk_sb, "kt", x8_c, half, kt)
            pw_box = {}
            for half in range(2):
                for part in range(2):
                    proj_w_half(wv_sb, x_c, half, vt, part, pw_box)
            v_sb = vpool.tile([128, KC, 128], BF16, name="v_sb", tag="v")
            transpose_v_chunk(vt, v_sb, 0, KC)
            return qt, kt, v_sb

        def scores_exp(b, kc, qt, kt, v_sb):
            """Scores + softmax-numerator + denominators + scaled V for
            key-chunk kc.  Each (h, half) unit runs its exponential on the
            engine given by the plan."""
            a_ts = [
                apool.tile([128, S], BF16, name=f"a{h}", tag=f"a{h}")
                for h in range(2)
            ]
            accs = [[], []]
            for half in range(2):
                tiles = [
                    ps.tile([128, 1024], F32, name=f"psc{h}", tag="work",
                            bufs=4)
                    for h in range(2)
                ]
                for qq in range(2):
                    q0 = half * 1024 + qq * 512
                    for h in range(2):
                        hp = h * 64
                        nc.tensor.matmul(
                            tiles[h][:, qq * 512:(qq + 1) * 512],
                            lhsT=kt[hp:hp + 64, kc * 128:(kc + 1) * 128],
                            rhs=qt[hp:hp + 64, q0:q0 + 512],
                            start=True,
                            stop=True,
                        )
                for h in range(2):
                    eng = plan[(kc, h, half)]
                    acc = small.tile([128, 1], F32, name="acc", tag="acc")
                    dst = a_ts[h][:, half * 1024:(half + 1) * 1024]
                    if eng == "a":
                        nc.scalar.activation(
                            dst, tiles[h], EXP,
                            scale=float(SCALE),
                            accum_out=acc,
                        )
                    else:
                        E = nc.vector if eng == "v" else nc.gpsimd
                        t = tpool.tile([128, 1024], BF16, name="t",
                                       tag=f"t{eng}")
                        # t = 1 + s/2  (s = raw * SCALE)
                        E.tensor_scalar(t, tiles[h], float(SCALE / 2), 1.0,
                                        MULT, ADD)
                        # a = t*t, denominator partial fused into accum
                        E.scalar_tensor_tensor(dst, t, 1.0, t, MULT, MULT,
                                               accum_out=acc)
                    accs[h].append(acc)
            res = []
            for h in range(2):
                den = small.tile([128, 1], F32, name="den", tag="den")
                nc.gpsimd.tensor_tensor(den, accs[h][0], accs[h][1], ADD)
                rec = small.tile([128, 1], F32, name="rec", tag="rec")
                nc.vector.reciprocal(rec, den)
                vs = small.tile([128, DH], BF16, name="vs", tag=f"vs{h}")
                nc.gpsimd.tensor_scalar(vs, v_sb[:, kc, h * 64:h * 64 + 64],
                                        rec, None, MULT)
                res.append((a_ts[h], vs))
            return res

        def av_pair(units, zac, first):
            """AV for two kc units: dense 16-matmul burst into two PSUM
            tiles (accumulating over the 2 kc), then fold into the SBUF
            f32 accumulator on DVE."""
            zps = [
                ps.tile([128, 1024], F32, name=f"zp{q2}", tag="work", bufs=4)
                for q2 in range(2)
            ]
            last = len(units) - 1
            for j, (kc, pair) in enumerate(units):
                for qc in range(4):
                    for h in range(2):
                        a_t, vs = pair[h]
                        hp = h * 64
                        nc.tensor.matmul(
                            zps[qc // 2][hp:hp + 64,
                                         (qc % 2) * 512:(qc % 2 + 1) * 512],
                            lhsT=vs,
                            rhs=a_t[:, qc * 512:(qc + 1) * 512],
                            start=(j == 0),
                            stop=(j == last),
                            skip_group_check=True,
                        )
            for q2 in range(2):
                sl = zac[:, q2 * 1024:(q2 + 1) * 1024]
                if first:
                    nc.vector.tensor_copy(sl, zps[q2])
                else:
                    nc.vector.tensor_add(sl, zps[q2], sl)

        def outproj_load_half(b, half):
            """Load the gathered z^T slabs for one half of batch b."""
            zf_c = []
            for j in range(4):
                zf = zslab.tile([128, 2, S // 2], BF16, name=f"zf{j}",
                                tag=f"zf{j}")
                nc.gpsimd.dma_start(
                    zf,
                    zfull[2 * b + half][j * 256:(j + 1) * 256, :]
                    .rearrange("(c p) s -> p c s", p=128),
                )
                zf_c.append(zf)
            return zf_c

        def outproj_chunk(b, half, part, zf_c, po_box):
            """8 of the 16 out-projection matmuls for one half; the second
            part evacuates PSUM via DVE and fires the output DMA."""
            if part == 0:
                po_box[half] = ps.tile([128, 1024], F32, name="po",
                                       tag="work", bufs=4)
            po = po_box[half]
            for lc in range(part * 4, part * 4 + 4):
                for sc in range(2):
                    nc.tensor.matmul(
                        po[:, sc * 512:(sc + 1) * 512],
                        lhsT=wo_sb[:, lc, :],
                        rhs=zf_c[lc // 2][:, lc % 2,
                                          sc * 512:(sc + 1) * 512],
                        start=(lc == 0),
                        stop=(lc == 7),
                        skip_group_check=True,
                    )
            if part == 1:
                o_sb = osb_p.tile([128, S // 2], F32, name="o_sb", tag="osb")
                nc.vector.tensor_copy(o_sb, po)
                nc.sync.dma_start(
                    outT[:, b * S + half * 1024:b * S + (half + 1) * 1024],
                    o_sb)

        def attention(b, cur, nxt_b):
            """Attention for batch b with the previous batch's out-projection
            and the next batch's loads/projections interleaved as small
            chunks so no engine sees a long burst."""
            qt, kt, v_sb = cur
            zac = ztp.tile([128, S], F32, name="zac", tag="zac")
            hooks = defaultdict(list)
            st = {"zf": {}, "po": {}, "pw": {}, "nxt": {}}

            if b >= 1:
                pb = b - 1
                hooks[1].append(lambda: st["zf"].__setitem__(
                    0, outproj_load_half(pb, 0)))
                hooks[2].append(lambda: st["zf"].__setitem__(
                    1, outproj_load_half(pb, 1)))
                for i, (half, part) in enumerate(
                        [(0, 0), (0, 1), (1, 0), (1, 1)]):
                    hooks[3 + i].append(
                        lambda half=half, part=part: outproj_chunk(
                            pb, half, part, st["zf"][half], st["po"]))
            if nxt_b is not None:
                nb = nxt_b
                n = st["nxt"]
                hooks[2].append(lambda: n.__setitem__("x8", load_x8(nb)))
                hooks[3].append(lambda: n.__setitem__("x", load_xT(nb)))
                hooks[5].append(lambda: n.__setitem__(
                    "qt", qk.tile([128, S], BF16, name="qt", tag="qt")))
                for half in range(2):
                    hooks[5 + half].append(
                        lambda half=half: proj_w8_half(
                            wq_sb, "qt", n["x8"], half, n["qt"]))
                hooks[7].append(lambda: n.__setitem__(
                    "kt", qk.tile([128, S], BF16, name="kt", tag="kt")))
                for half in range(2):
                    hooks[7 + half].append(
                        lambda half=half: proj_w8_half(
                            wk_sb, "kt", n["x8"], half, n["kt"]))
                hooks[9].append(lambda: n.__setitem__(
                    "vt", vtpool.tile([128, S], BF16, name="vt", tag="vt")))
                for i, (half, part) in enumerate(
                        [(0, 0), (0, 1), (1, 0), (1, 1)]):
                    hooks[9 + i].append(
                        lambda half=half, part=part: proj_w_half(
                            wv_sb, n["x"], half, n["vt"], part, st["pw"]))
                hooks[13].append(lambda: n.__setitem__(
                    "v", vpool.tile([128, KC, 128], BF16, name="v_sb",
                                    tag="v")))
                hooks[13].append(
                    lambda: transpose_v_chunk(n["vt"], n["v"], 0, 8))
                hooks[14].append(
                    lambda: transpose_v_chunk(n["vt"], n["v"], 8, KC))

            pending = []
            npairs = 0
            for kc in range(KC):
                pending.append((kc, scores_exp(b, kc, qt, kt, v_sb)))
                thr = 6 if kc < 12 else 4
                if len(pending) >= thr:
                    av_pair(pending[:2], zac, first=(npairs == 0))
                    pending = pending[2:]
                    npairs += 1
                for fn in hooks.get(kc, []):
                    fn()
            while pending:
                av_pair(pending[:2], zac, first=(npairs == 0))
                pending = pending[2:]
                npairs += 1
            # flush + AllGather per s-half (f32 -> bf16 cast inside the DMA)
            for half in range(2):
                nc.gpsimd.dma_start(
                    zloc[b, half], zac[:, half * 1024:(half + 1) * 1024])
                nc.gpsimd.collective_compute(
                    "AllGather",
                    mybir.AluOpType.bypass,
                    replica_groups=[list(range(NCORES))],
                    ins=[zloc[b, half].opt()],
                    outs=[zfull[2 * b + half][:, :].opt()],
                )
            if nxt_b is not None:
                return st["nxt"]["qt"], st["nxt"]["kt"], st["nxt"]["v"]
            return None

        cur = proj(0)
        for b in range(B):
            cur = attention(b, cur, b + 1 if b < B - 1 else None)
        # tail: out-projection of the last batch (burst form)
        zf_t = [outproj_load_half(B - 1, half) for half in range(2)]
        po_box = {}
        for half in range(2):
            for part in range(2):
                outproj_chunk(B - 1, half, part, zf_t[half], po_box)


def _legalize_waits(nc):
    """This walrus build accepts only ~2 sync commands (1 wait + 1 inc) per
    instruction for the standard engine/DMA templates; Tile can emit 2-3
    waits (WAR + WAW + RAW). Hoist all but one wait of any multi-wait
    instruction onto single-wait NOPs on the same engine, immediately
    before it — the raw-bass `wait_ge; op` pattern. Drain/EventSemaphore
    templates accept many waits (the kernel-tail barrier relies on it)."""
    import bass_rust

    n = 0
    for f in nc.m.functions:
        for blk in f.blocks:
            out = []
            changed = False
            for inst in blk.instructions:
                si = inst.sync_info
                if si is not None and len(si.on_wait) > 1:
                    for w in si.on_wait[:-1]:
                        n += 1
                        out.append(
                            bass_rust.InstNoOp(
                                name=f"I-hoistwait-{n}",
                                engine=inst.engine,
                                bass_nofuse=True,
                                sync_info=bass_rust.SyncInfo(
                                    on_wait=[w], on_update=[]
                                ),
                            )
                        )
                    inst.sync_info = bass_rust.SyncInfo(
                        on_wait=[si.on_wait[-1]], on_update=list(si.on_update)
                    )
                    changed = True
                out.append(inst)
            if changed:
                blk.instructions = out


def build(legalize=True, n_dve=0, n_pool=0):
    nc = bass.Bass(
        "TRN2",
        target_bir_lowering=False,
        debug=False,
        enable_asserts=False,
        num_devices=NCORES,
    )
    xT = nc.dram_tensor("xT", [B, D, S], BF16, kind="ExternalInput").ap()
    x8 = nc.dram_tensor("x8", [B, 4, 128, 2, S], F8, kind="ExternalInput").ap()
    wq8 = nc.dram_tensor("wq8", [128, 4, 2, LPC], F8, kind="ExternalInput").ap()
    wk8 = nc.dram_tensor("wk8", [128, 4, 2, LPC], F8, kind="ExternalInput").ap()
    wvT = nc.dram_tensor("wvT", [D, LPC], BF16, kind="ExternalInput").ap()
    woT = nc.dram_tensor("woT", [L, DPC], BF16, kind="ExternalInput").ap()
    outT = nc.dram_tensor("outT", [DPC, B * S], F32, kind="ExternalOutput").ap()

    plan = _default_unit_plan(n_dve, n_pool)

    with tile.TileContext(nc) as tc:
        from contextlib import ExitStack

        with ExitStack() as ctx:
            dram = ctx.enter_context(tc.tile_pool(name="dram", bufs=1, space="DRAM"))
            zloc = dram.tile([B, 2, LPC, S // 2], BF16, name="zloc")
            zfull = [
                dram.tile([L, S // 2], BF16, name=f"zfull{i}", tag=f"zfull{i}",
                          addr_space="Shared")
                for i in range(2 * B)
            ]
            _body(tc, xT, x8, wq8, wk8, wvT, woT, outT, zloc, zfull, plan)
    if legalize:
        # the inserted NOPs are invisible to the simulator's race-detector
        # registry; sim callers pass legalize=False (identical semantics)
        _legalize_waits(nc)
    return nc


def make_in_maps(x, Wq, Wk, Wv, Wo):
    bf = ml_dtypes.bfloat16
    f8 = ml_dtypes.float8_e4m3
    x = np.asarray(x, np.float32)
    xTf = np.ascontiguousarray(x.transpose(0, 2, 1))            # (B, D, S)
    xT = xTf.astype(bf)
    # fp8 copy with D-chunk pairs interleaved for DoubleRow matmuls
    x8 = np.ascontiguousarray(
        xTf.reshape(B, 4, 2, 128, S).transpose(0, 1, 3, 2, 4)).astype(f8)
    WoT = np.ascontiguousarray(np.asarray(Wo, np.float32).T)    # (L, D)

    def w8(W, rs):
        wT = np.asarray(W, np.float32)[rs].T                    # (D, 128)
        return np.ascontiguousarray(
            wT.reshape(4, 2, 128, LPC).transpose(2, 0, 1, 3)).astype(f8)

    in_maps = []
    for c in range(NCORES):
        rs = slice(128 * c, 128 * (c + 1))
        in_maps.append({
            "xT": xT,
            "x8": x8,
            "wq8": w8(Wq, rs),
            "wk8": w8(Wk, rs),
            "wvT": np.ascontiguousarray(np.asarray(Wv, np.float32)[rs].T).astype(bf),
            "woT": np.ascontiguousarray(WoT[:, rs]).astype(bf),
        })
    return in_maps


def _install_ntff_hook_shim():
    """This container's `antenv` lacks `axon_hooks`; recreate the NTFF
    profile hook (same ctypes recipe as trn_agent_boot.trn_boot) so
    run_bass_kernel_spmd(trace=True) can capture exec_time_ns."""
    import sys
    import types
    import ctypes
    import contextlib

    try:
        import antenv.axon_hooks  # noqa: F401
        return
    except ImportError:
        pass

    hook = None
    so_path = os.environ.get("PJRT_LIBRARY_PATH")
    if so_path and os.path.exists(so_path):
        try:
            lib = ctypes.CDLL(so_path)
            if hasattr(lib, "axon_start_nrt_profile"):
                lib.axon_start_nrt_profile.argtypes = [
                    ctypes.POINTER(ctypes.c_int64),
                    ctypes.c_size_t,
                ]
                lib.axon_start_nrt_profile.restype = ctypes.c_int64
                lib.axon_stop_nrt_profile.argtypes = [ctypes.c_char_p]
                lib.axon_stop_nrt_profile.restype = ctypes.c_int64

                @contextlib.contextmanager
                def _hook(output_dir, device_ids):
                    import jax

                    jax.devices()
                    if device_ids:
                        ids = (ctypes.c_int64 * len(device_ids))(*device_ids)
                        rc = lib.axon_start_nrt_profile(ids, len(device_ids))
                    else:
                        rc = lib.axon_start_nrt_profile(None, 0)
                    if rc != 0:
                        raise RuntimeError(f"axon_start_nrt_profile rc={rc}")
                    try:
                        yield
                    finally:
                        n = lib.axon_stop_nrt_profile(str(output_dir).encode())
                        print(f"profile: {n} file(s) written to {output_dir}")

                hook = _hook
        except OSError:
            hook = None

    mod = types.ModuleType("antenv.axon_hooks")
    mod.get_axon_ntff_profile_hook = lambda: hook
    mod.set_axon_ntff_profile_hook = lambda h: None
    sys.modules["antenv.axon_hooks"] = mod
    import antenv

    antenv.axon_hooks = mod


def _gather(res):
    return np.concatenate(
        [np.asarray(res.results[c]["outT"], np.float32) for c in range(NCORES)],
        axis=0,
    )  # (D, B*S)


N_DVE = int(os.environ.get("KERNEL_N_DVE", "0"))
N_POOL = int(os.environ.get("KERNEL_N_POOL", "0"))


def kernel(x, Wq, Wk, Wv, Wo):
    global LAST_EXEC_NS
    in_maps = make_in_maps(x, Wq, Wk, Wv, Wo)
    nc = build(n_dve=N_DVE, n_pool=N_POOL)
    trace = bool(int(os.environ.get("BASS_KERNEL_TRACE", "0")))
    if trace:
        _install_ntff_hook_shim()
    core_ids = list(range(NCORES))
    # Run twice and cross-check: the first execution of a freshly-loaded
    # NEFF was once observed to produce a corrupted result; a re-run is
    # ~0.6ms of device time against a multi-second compile+load.
    r1 = _gather(run_bass_kernel_spmd(nc, in_maps, core_ids=core_ids))
    res = run_bass_kernel_spmd(nc, in_maps, core_ids=core_ids, trace=trace)
    LAST_EXEC_NS = res.exec_time_ns
    r2 = _gather(res)
    if not np.array_equal(r1, r2):
        r3 = _gather(run_bass_kernel_spmd(nc, in_maps, core_ids=core_ids))
        outT = r3 if np.array_equal(r2, r3) else (
            r1 if np.array_equal(r1, r3) else r2)
    else:
        outT = r2
    return np.ascontiguousarray(outT.T).reshape(B, S, D).astype(np.float32)


# revision 8
# speedup vs baseline: 1.1197x; 1.1197x over previous
"""Trainium2 Bass kernel for nn_Attention_9431748182617.

Quirky attention: scores z[b,k,q] = (q_h . k_h) / sqrt(D), softmax over the
QUERY axis (per key row), out[q] = sum_k A[k,q] * v[k], then output projection.

Sharding (8 NeuronCores):
  - tensor-parallel over heads: 16 heads -> 2 heads per core.
    Each core owns rows [128c, 128c+128) of Wq/Wk/Wv (its 2 heads) and
    computes q/k/v + attention for those heads over the full batch.
  - z^T (local 128 rows of L, all of B*S) is AllGather'd per batch.
  - output projection sharded by output feature D: core c computes
    out^T rows [128c, 128c+128) using Wo^T[:, 128c:128c+128] for ALL s.
  - host concatenates the 8 out^T blocks and transposes.

The Scalar engine (exp over B*H*S^2/8 = 33.5M elems/core) is the critical
resource; all other work is issued as <=8-matmul chunks interleaved between
score units so ScalarE never stalls.  A tunable subset of softmax units runs
as (1+s/2)^2 on DVE/Pool (scores are |s|<0.7 so the quadratic is ~0.4% err)
with the denominator fused into the last op's accumulator, offloading ACT.
Matmuls in bf16 (fp32 PSUM accumulation), Q/K projections fp8 DoubleRow.
"""

import os
from collections import defaultdict

import numpy as np
import ml_dtypes

import concourse.bass as bass
import concourse.mybir as mybir
import concourse.tile as tile
from concourse.bass_utils import run_bass_kernel_spmd
from concourse.masks import make_identity

B, S, D = 4, 2048, 1024
L, H = 1024, 16
DH = L // H               # 64
NCORES = 8
LPC = L // NCORES         # 128 l-rows (= 2 heads) per core
DPC = D // NCORES         # 128 out-feature rows per core
SCALE = 1.0 / (D ** 0.5)
KC = S // 128             # 16 key chunks of 128
BF16 = mybir.dt.bfloat16
F32 = mybir.dt.float32
F8 = mybir.dt.float8e4
EXP = mybir.ActivationFunctionType.Exp
MULT = mybir.AluOpType.mult
ADD = mybir.AluOpType.add

LAST_EXEC_NS = None

# softmax-unit engine assignment: for each (kc, h, half) unit, which engine
# computes the exponentials. "a"=ScalarE exp, "v"=DVE (1+s/2)^2, "p"=Pool.
# 64 units per batch; same pattern every batch.
UNIT_ENGINE = {}


def _default_unit_plan(n_dve=0, n_pool=0):
    """Distribute n_dve DVE units + n_pool Pool units evenly over the
    64 (kc, h, half) units of a batch; the rest stay on ScalarE."""
    plan = {}
    units = [(kc, h, half) for kc in range(KC) for h in range(2)
             for half in range(2)]
    n = len(units)
    picks = {}
    for cnt, tag in ((n_dve, "v"), (n_pool, "p")):
        step = n / max(cnt, 1)
        k = 0.5
        placed = 0
        while placed < cnt:
            i = int(k) % n
            while units[i] in picks:
                i = (i + 1) % n
            picks[units[i]] = tag
            placed += 1
            k += step
    for u in units:
        plan[u] = picks.get(u, "a")
    return plan


def _body(tc, xT, x8, wq8, wk8, wvT, woT, outT, zloc, zfull, plan):
    nc = tc.nc
    from contextlib import ExitStack

    with ExitStack() as ctx:
        const = ctx.enter_context(tc.tile_pool(name="const", bufs=1))
        xpool = ctx.enter_context(tc.tile_pool(name="xpool", bufs=1))
        qk = ctx.enter_context(tc.tile_pool(name="qk", bufs=2))
        vtpool = ctx.enter_context(tc.tile_pool(name="vtpool", bufs=1))
        vpool = ctx.enter_context(tc.tile_pool(name="vpool", bufs=2))
        apool = ctx.enter_context(tc.tile_pool(name="apool", bufs=7))
        tpool = ctx.enter_context(tc.tile_pool(name="tpool", bufs=4))
        small = ctx.enter_context(tc.tile_pool(name="small", bufs=8))
        ztp = ctx.enter_context(tc.tile_pool(name="ztp", bufs=2))
        zslab = ctx.enter_context(tc.tile_pool(name="zslab", bufs=2))
        osb_p = ctx.enter_context(tc.tile_pool(name="osb_p", bufs=2))
        # PSUM split: a 3-deep scores-only ring feeding the softmax engines
        # (so no slow consumer can ever stall ScalarE through pool-FIFO
        # coupling) and one aux tile cycled by AV partials / projections /
        # out-projection, whose slow deps only ever delay AV (which has
        # slack).  3*2 + 2 banks = all 8.
        ps = ctx.enter_context(tc.tile_pool(name="ps", bufs=1, space="PSUM"))

        # ---- constants: weights ----
        wq_sb = const.tile([128, 4, 2, 128], F8, name="wq_sb")
        wk_sb = const.tile([128, 4, 2, 128], F8, name="wk_sb")
        nc.sync.dma_start(wq_sb, wq8)
        nc.sync.dma_start(wk_sb, wk8)
        wv_sb = const.tile([128, 8, 128], BF16, name="wv_sb")
        wo_sb = const.tile([128, 8, 128], BF16, name="wo_sb")
        for dc in range(8):
            nc.sync.dma_start(wv_sb[:, dc, :], wvT[dc * 128:(dc + 1) * 128, :])
            nc.sync.dma_start(wo_sb[:, dc, :], woT[dc * 128:(dc + 1) * 128, :])
        # fire the exp table load (~2.7us) under the startup DMAs
        warm_in = const.tile([128, 1], F32, name="warm_in")
        warm_out = const.tile([128, 1], F32, name="warm_out")
        nc.vector.memset(warm_in, 0.0)
        nc.scalar.activation(warm_out, warm_in, EXP)

        def load_x8(b):
            x8_c = []
            for j in range(4):
                xc = xpool.tile([128, 2, S], F8, name=f"x8c{j}", tag=f"x8{j}")
                nc.gpsimd.dma_start(xc, x8[b, j])
                x8_c.append(xc)
            return x8_c

        def load_xT(b):
            x_c = []
            for dc in range(8):
                xc = xpool.tile([128, S], BF16, name=f"xc{dc}", tag=f"x{dc}")
                nc.gpsimd.dma_start(xc, xT[b, dc * 128:(dc + 1) * 128, :])
                x_c.append(xc)
            return x_c

        def proj_w8_half(w_sb, nm, x8_c, half, dest):
            """One s-half of a Q/K projection in fp8 DoubleRow (8 MMs)."""
            pw = ps.tile([128, 1024], F32, name="pw8", tag="aux", bufs=1)
            for j in range(4):
                for q in range(2):
                    sc = half * 2 + q
                    nc.tensor.matmul(
                        pw[:, q * 512:(q + 1) * 512],
                        lhsT=w_sb[:, j, :, :],
                        rhs=x8_c[j][:, :, sc * 512:(sc + 1) * 512],
                        start=(j == 0),
                        stop=(j == 3),
                        perf_mode=mybir.MatmulPerfMode.DoubleRow,
                    )
            nc.vector.tensor_copy(dest[:, half * 1024:(half + 1) * 1024], pw)

        def proj_w_half(x_c, half, dest):
            """One s-half of the V projection (16 MMs + evacuation)."""
            pw = ps.tile([128, 1024], F32, name="pwv", tag="aux", bufs=1)
            for dc in range(8):
                for q in range(2):
                    sc = half * 2 + q
                    nc.tensor.matmul(
                        pw[:, q * 512:(q + 1) * 512],
                        lhsT=wv_sb[:, dc, :],
                        rhs=x_c[dc][:, sc * 512:(sc + 1) * 512],
                        start=(dc == 0),
                        stop=(dc == 7),
                    )
            nc.vector.tensor_copy(dest[:, half * 1024:(half + 1) * 1024],
                                  pw)

        def transpose_v_chunk(vt, v_sb, c0, c1):
            for c in range(c0, c1):
                nc.sync.dma_start_transpose(
                    v_sb[:, c, :], vt[:, c * 128:(c + 1) * 128])

        def proj(b):
            """Initial (unhooked) projection burst for batch 0."""
            x8_c = load_x8(b)
            x_c = load_xT(b)
            qt = qk.tile([128, S], BF16, name="qt", tag="qt")
            kt = qk.tile([128, S], BF16, name="kt", tag="kt")
            vt = vtpool.tile([128, S], BF16, name="vt", tag="vt")
            for half in range(2):
                proj_w8_half(wq_sb, "qt", x8_c, half, qt)
            for half in range(2):
                proj_w8_half(wk_sb, "kt", x8_c, half, kt)
            for half in range(2):
                proj_w_half(x_c, half, vt)
            v_sb = vpool.tile([128, KC, 128], BF16, name="v_sb", tag="v")
            transpose_v_chunk(vt, v_sb, 0, KC)
            return qt, kt, v_sb

        def scores_exp(b, kc, qt, kt, v_sb):
            """Scores + softmax-numerator + denominators + scaled V for
            key-chunk kc.  Each (h, half) unit runs its exponential on the
            engine given by the plan."""
            a_ts = [
                apool.tile([128, S], BF16, name=f"a{h}", tag=f"a{h}")
                for h in range(2)
            ]
            accs = [[], []]
            for half in range(2):
                tiles = [
                    ps.tile([128, 1024], F32, name=f"psc{h}", tag="sc",
                            bufs=3)
                    for h in range(2)
                ]
                for qq in range(2):
                    q0 = half * 1024 + qq * 512
                    for h in range(2):
                        hp = h * 64
                        nc.tensor.matmul(
                            tiles[h][:, qq * 512:(qq + 1) * 512],
                            lhsT=kt[hp:hp + 64, kc * 128:(kc + 1) * 128],
                            rhs=qt[hp:hp + 64, q0:q0 + 512],
                            start=True,
                            stop=True,
                        )
                for h in range(2):
                    eng = plan[(kc, h, half)]
                    acc = small.tile([128, 1], F32, name="acc", tag="acc")
                    dst = a_ts[h][:, half * 1024:(half + 1) * 1024]
                    if eng == "a":
                        nc.scalar.activation(
                            dst, tiles[h], EXP,
                            scale=float(SCALE),
                            accum_out=acc,
                        )
                    else:
                        E = nc.vector if eng == "v" else nc.gpsimd
                        t = tpool.tile([128, 1024], BF16, name="t",
                                       tag=f"t{eng}")
                        # t = 1 + s/2  (s = raw * SCALE)
                        E.tensor_scalar(t, tiles[h], float(SCALE / 2), 1.0,
                                        MULT, ADD)
                        # a = t*t, denominator partial fused into accum
                        E.scalar_tensor_tensor(dst, t, 1.0, t, MULT, MULT,
                                               accum_out=acc)
                    accs[h].append(acc)
            res = []
            for h in range(2):
                den = small.tile([128, 1], F32, name="den", tag="den")
                nc.gpsimd.tensor_tensor(den, accs[h][0], accs[h][1], ADD)
                rec = small.tile([128, 1], F32, name="rec", tag="rec")
                nc.vector.reciprocal(rec, den)
                vs = small.tile([128, DH], BF16, name="vs", tag=f"vs{h}")
                nc.gpsimd.tensor_scalar(vs, v_sb[:, kc, h * 64:h * 64 + 64],
                                        rec, None, MULT)
                res.append((a_ts[h], vs))
            return res

        def av_pair(units, zac, first):
            """AV for two kc units: per q-half an 8-matmul burst into the
            aux PSUM tile (accumulating over the 2 kc), folded into the
            SBUF f32 accumulator on DVE."""
            last = len(units) - 1
            for q2 in range(2):
                zp = ps.tile([128, 1024], F32, name="zp", tag="aux", bufs=1)
                for j, (kc, pair) in enumerate(units):
                    for qq in range(2):
                        qc = q2 * 2 + qq
                        for h in range(2):
                            a_t, vs = pair[h]
                            hp = h * 64
                            nc.tensor.matmul(
                                zp[hp:hp + 64, qq * 512:(qq + 1) * 512],
                                lhsT=vs,
                                rhs=a_t[:, qc * 512:(qc + 1) * 512],
                                start=(j == 0),
                                stop=(j == last),
                                skip_group_check=True,
                            )
                sl = zac[:, q2 * 1024:(q2 + 1) * 1024]
                if first:
                    nc.vector.tensor_copy(sl, zp)
                else:
                    nc.vector.tensor_add(sl, zp, sl)

        def outproj_load_half(b, half):
            """Load the gathered z^T slabs for one half of batch b."""
            zf_c = []
            for j in range(4):
                zf = zslab.tile([128, 2, S // 2], BF16, name=f"zf{j}",
                                tag=f"zf{j}")
                nc.sync.dma_start(
                    zf,
                    zfull[2 * b + half][j * 256:(j + 1) * 256, :]
                    .rearrange("(c p) s -> p c s", p=128),
                )
                zf_c.append(zf)
            return zf_c

        def outproj_half(b, half, zf_c):
            """One half of the out-projection: 16 MMs + evacuation + DMA."""
            po = ps.tile([128, 1024], F32, name="po", tag="aux", bufs=1)
            for lc in range(8):
                for sc in range(2):
                    nc.tensor.matmul(
                        po[:, sc * 512:(sc + 1) * 512],
                        lhsT=wo_sb[:, lc, :],
                        rhs=zf_c[lc // 2][:, lc % 2,
                                          sc * 512:(sc + 1) * 512],
                        start=(lc == 0),
                        stop=(lc == 7),
                        skip_group_check=True,
                    )
            o_sb = osb_p.tile([128, S // 2], F32, name="o_sb", tag="osb")
            nc.vector.tensor_copy(o_sb, po)
            nc.sync.dma_start(
                outT[:, b * S + half * 1024:b * S + (half + 1) * 1024],
                o_sb)

        def attention(b, cur, nxt_b):
            """Attention for batch b with the previous batch's out-projection
            and the next batch's loads/projections interleaved as small
            chunks so no engine sees a long burst."""
            qt, kt, v_sb = cur
            zac = ztp.tile([128, S], F32, name="zac", tag="zac")
            hooks = defaultdict(list)
            st = {"zf": {}, "nxt": {}}

            if nxt_b is not None:
                nb = nxt_b
                n = st["nxt"]
                hooks[1].append(lambda: n.__setitem__("x8", load_x8(nb)))
                hooks[2].append(lambda: n.__setitem__("x", load_xT(nb)))
                hooks[5].append(lambda: n.__setitem__(
                    "qt", qk.tile([128, S], BF16, name="qt", tag="qt")))
                for half in range(2):
                    hooks[5 + half].append(
                        lambda half=half: proj_w8_half(
                            wq_sb, "qt", n["x8"], half, n["qt"]))
                hooks[7].append(lambda: n.__setitem__(
                    "kt", qk.tile([128, S], BF16, name="kt", tag="kt")))
                for half in range(2):
                    hooks[7 + half].append(
                        lambda half=half: proj_w8_half(
                            wk_sb, "kt", n["x8"], half, n["kt"]))
                hooks[10].append(lambda: n.__setitem__(
                    "vt", vtpool.tile([128, S], BF16, name="vt", tag="vt")))
                for half in range(2):
                    hooks[10 + 2 * half].append(
                        lambda half=half: proj_w_half(
                            n["x"], half, n["vt"]))
                hooks[13].append(lambda: n.__setitem__(
                    "v", vpool.tile([128, KC, 128], BF16, name="v_sb",
                                    tag="v")))
                hooks[13].append(
                    lambda: transpose_v_chunk(n["vt"], n["v"], 0, 8))
                hooks[14].append(
                    lambda: transpose_v_chunk(n["vt"], n["v"], 8, KC))
            if b >= 1:
                pb = b - 1
                hooks[2].append(lambda: st["zf"].__setitem__(
                    0, outproj_load_half(pb, 0)))
                hooks[3].append(lambda: st["zf"].__setitem__(
                    1, outproj_load_half(pb, 1)))
                # out-projection halves go LAST in the batch's aux-pool
                # order: their matmuls wait on the gathered slabs, and any
                # aux user queued behind them would inherit that wait.
                hooks[12].append(lambda: outproj_half(pb, 0, st["zf"][0]))
                hooks[14].append(lambda: outproj_half(pb, 1, st["zf"][1]))

            pending = []
            npairs = 0
            for kc in range(KC):
                pending.append((kc, scores_exp(b, kc, qt, kt, v_sb)))
                for fn in hooks.get(kc, []):
                    fn()
                thr = 6 if kc < 12 else 4
                if len(pending) >= thr:
                    av_pair(pending[:2], zac, first=(npairs == 0))
                    pending = pending[2:]
                    npairs += 1
            while pending:
                av_pair(pending[:2], zac, first=(npairs == 0))
                pending = pending[2:]
                npairs += 1
            # flush + AllGather per s-half (f32 -> bf16 cast inside the DMA)
            for half in range(2):
                nc.gpsimd.dma_start(
                    zloc[b, half], zac[:, half * 1024:(half + 1) * 1024])
                nc.gpsimd.collective_compute(
                    "AllGather",
                    mybir.AluOpType.bypass,
                    replica_groups=[list(range(NCORES))],
                    ins=[zloc[b, half].opt()],
                    outs=[zfull[2 * b + half][:, :].opt()],
                )
            if nxt_b is not None:
                return st["nxt"]["qt"], st["nxt"]["kt"], st["nxt"]["v"]
            return None

        cur = proj(0)
        for b in range(B):
            cur = attention(b, cur, b + 1 if b < B - 1 else None)
        # tail: out-projection of the last batch (burst form)
        for half in range(2):
            zf_t = outproj_load_half(B - 1, half)
            outproj_half(B - 1, half, zf_t)


def _legalize_waits(nc):
    """This walrus build accepts only ~2 sync commands (1 wait + 1 inc) per
    instruction for the standard engine/DMA templates; Tile can emit 2-3
    waits (WAR + WAW + RAW). Hoist all but one wait of any multi-wait
    instruction onto single-wait NOPs on the same engine, immediately
    before it — the raw-bass `wait_ge; op` pattern. Drain/EventSemaphore
    templates accept many waits (the kernel-tail barrier relies on it)."""
    import bass_rust

    n = 0
    for f in nc.m.functions:
        for blk in f.blocks:
            out = []
            changed = False
            for inst in blk.instructions:
                si = inst.sync_info
                if si is not None and len(si.on_wait) > 1:
                    for w in si.on_wait[:-1]:
                        n += 1
                        out.append(
                            bass_rust.InstNoOp(
                                name=f"I-hoistwait-{n}",
                                engine=inst.engine,
                                bass_nofuse=True,
                                sync_info=bass_rust.SyncInfo(
                                    on_wait=[w], on_update=[]
                                ),
                            )
                        )
                    inst.sync_info = bass_rust.SyncInfo(
                        on_wait=[si.on_wait[-1]], on_update=list(si.on_update)
                    )
                    changed = True
                out.append(inst)
            if changed:
                blk.instructions = out


def build(legalize=True, n_dve=0, n_pool=0):
    nc = bass.Bass(
        "TRN2",
        target_bir_lowering=False,
        debug=False,
        enable_asserts=False,
        num_devices=NCORES,
    )
    xT = nc.dram_tensor("xT", [B, D, S], BF16, kind="ExternalInput").ap()
    x8 = nc.dram_tensor("x8", [B, 4, 128, 2, S], F8, kind="ExternalInput").ap()
    wq8 = nc.dram_tensor("wq8", [128, 4, 2, LPC], F8, kind="ExternalInput").ap()
    wk8 = nc.dram_tensor("wk8", [128, 4, 2, LPC], F8, kind="ExternalInput").ap()
    wvT = nc.dram_tensor("wvT", [D, LPC], BF16, kind="ExternalInput").ap()
    woT = nc.dram_tensor("woT", [L, DPC], BF16, kind="ExternalInput").ap()
    outT = nc.dram_tensor("outT", [DPC, B * S], F32, kind="ExternalOutput").ap()

    plan = _default_unit_plan(n_dve, n_pool)

    with tile.TileContext(nc) as tc:
        from contextlib import ExitStack

        with ExitStack() as ctx:
            dram = ctx.enter_context(tc.tile_pool(name="dram", bufs=1, space="DRAM"))
            zloc = dram.tile([B, 2, LPC, S // 2], BF16, name="zloc")
            zfull = [
                dram.tile([L, S // 2], BF16, name=f"zfull{i}", tag=f"zfull{i}",
                          addr_space="Shared")
                for i in range(2 * B)
            ]
            _body(tc, xT, x8, wq8, wk8, wvT, woT, outT, zloc, zfull, plan)
    if legalize:
        # the inserted NOPs are invisible to the simulator's race-detector
        # registry; sim callers pass legalize=False (identical semantics)
        _legalize_waits(nc)
    return nc


def make_in_maps(x, Wq, Wk, Wv, Wo):
    bf = ml_dtypes.bfloat16
    f8 = ml_dtypes.float8_e4m3
    x = np.asarray(x, np.float32)
    xTf = np.ascontiguousarray(x.transpose(0, 2, 1))            # (B, D, S)
    xT = xTf.astype(bf)
    # fp8 copy with D-chunk pairs interleaved for DoubleRow matmuls
    x8 = np.ascontiguousarray(
        xTf.reshape(B, 4, 2, 128, S).transpose(0, 1, 3, 2, 4)).astype(f8)
    WoT = np.ascontiguousarray(np.asarray(Wo, np.float32).T)    # (L, D)

    def w8(W, rs):
        wT = np.asarray(W, np.float32)[rs].T                    # (D, 128)
        return np.ascontiguousarray(
            wT.reshape(4, 2, 128, LPC).transpose(2, 0, 1, 3)).astype(f8)

    in_maps = []
    for c in range(NCORES):
        rs = slice(128 * c, 128 * (c + 1))
        in_maps.append({
            "xT": xT,
            "x8": x8,
            "wq8": w8(Wq, rs),
            "wk8": w8(Wk, rs),
            "wvT": np.ascontiguousarray(np.asarray(Wv, np.float32)[rs].T).astype(bf),
            "woT": np.ascontiguousarray(WoT[:, rs]).astype(bf),
        })
    return in_maps


def _install_ntff_hook_shim():
    """This container's `antenv` lacks `axon_hooks`; recreate the NTFF
    profile hook (same ctypes recipe as trn_agent_boot.trn_boot) so
    run_bass_kernel_spmd(trace=True) can capture exec_time_ns."""
    import sys
    import types
    import ctypes
    import contextlib

    try:
        import antenv.axon_hooks  # noqa: F401
        return
    except ImportError:
        pass

    hook = None
    so_path = os.environ.get("PJRT_LIBRARY_PATH")
    if so_path and os.path.exists(so_path):
        try:
            lib = ctypes.CDLL(so_path)
            if hasattr(lib, "axon_start_nrt_profile"):
                lib.axon_start_nrt_profile.argtypes = [
                    ctypes.POINTER(ctypes.c_int64),
                    ctypes.c_size_t,
                ]
                lib.axon_start_nrt_profile.restype = ctypes.c_int64
                lib.axon_stop_nrt_profile.argtypes = [ctypes.c_char_p]
                lib.axon_stop_nrt_profile.restype = ctypes.c_int64

                @contextlib.contextmanager
                def _hook(output_dir, device_ids):
                    import jax

                    jax.devices()
                    if device_ids:
                        ids = (ctypes.c_int64 * len(device_ids))(*device_ids)
                        rc = lib.axon_start_nrt_profile(ids, len(device_ids))
                    else:
                        rc = lib.axon_start_nrt_profile(None, 0)
                    if rc != 0:
                        raise RuntimeError(f"axon_start_nrt_profile rc={rc}")
                    try:
                        yield
                    finally:
                        n = lib.axon_stop_nrt_profile(str(output_dir).encode())
                        print(f"profile: {n} file(s) written to {output_dir}")

                hook = _hook
        except OSError:
            hook = None

    mod = types.ModuleType("antenv.axon_hooks")
    mod.get_axon_ntff_profile_hook = lambda: hook
    mod.set_axon_ntff_profile_hook = lambda h: None
    sys.modules["antenv.axon_hooks"] = mod
    import antenv

    antenv.axon_hooks = mod


def _gather(res):
    return np.concatenate(
        [np.asarray(res.results[c]["outT"], np.float32) for c in range(NCORES)],
        axis=0,
    )  # (D, B*S)


N_DVE = int(os.environ.get("KERNEL_N_DVE", "0"))
N_POOL = int(os.environ.get("KERNEL_N_POOL", "0"))


def kernel(x, Wq, Wk, Wv, Wo):
    global LAST_EXEC_NS
    in_maps = make_in_maps(x, Wq, Wk, Wv, Wo)
    nc = build(n_dve=N_DVE, n_pool=N_POOL)
    trace = bool(int(os.environ.get("BASS_KERNEL_TRACE", "0")))
    if trace:
        _install_ntff_hook_shim()
    core_ids = list(range(NCORES))
    # Run twice and cross-check: the first execution of a freshly-loaded
    # NEFF was once observed to produce a corrupted result; a re-run is
    # ~0.6ms of device time against a multi-second compile+load.
    r1 = _gather(run_bass_kernel_spmd(nc, in_maps, core_ids=core_ids))
    res = run_bass_kernel_spmd(nc, in_maps, core_ids=core_ids, trace=trace)
    LAST_EXEC_NS = res.exec_time_ns
    r2 = _gather(res)
    if not np.array_equal(r1, r2):
        r3 = _gather(run_bass_kernel_spmd(nc, in_maps, core_ids=core_ids))
        outT = r3 if np.array_equal(r2, r3) else (
            r1 if np.array_equal(r1, r3) else r2)
    else:
        outT = r2
    return np.ascontiguousarray(outT.T).reshape(B, S, D).astype(np.float32)
